# revision 1
# baseline (speedup 1.0000x reference)
"""Multi-head attention (16 heads, D=1024, B=2, S=2048) on 8 TRN2 NeuronCores.

Sharding: tensor-parallel over heads. Each core owns 2 heads (128 features):
W_q/k/v column-sliced, W_o row-sliced; partial outputs summed on host.

Device dataflow (per core), everything kept "transposed" (features on
partitions) so the key-padding mask folds into the ACT exp bias and the
attention matrix is produced directly in the layout the A@V matmul needs:

  QT[f,s] = Wq_c^T @ x^T        (PE, contraction d on partitions)
  KT[f,s] likewise; VT -> PE-transpose -> V[s,f] (natural, k on partitions)
  scores^T[k,q] = KT_h^T-slice . QT_h  (row-tiled pairs, 2 heads)
  attn^T = exp(scores*0.125 + mask_bias[k])   (ACT, PSUM->SBUF, bf16)
  out_h^T[d,q] (+rowsum in row 64) = [V_h | 1]^T . attn_h^T  (PSUM accum over k)
  normalize: recip(rowsum) broadcast over partitions via tiny PE matmul
  out_part[s,:] = outT^T . Wo_c  -> fp16 partial, host sums in fp32

Key-padding mask: k-chunks entirely beyond valid_len are skipped (program is
specialized to the valid_lens values at call time); the boundary chunk uses a
-1e6 additive bias inside the exp activation (exp underflows to exact 0).
"""

import math
import os

import ml_dtypes
import numpy as np

B = 2
S = 2048
D = 1024
NT = B * S          # 4096 rows, b-major
F = 128             # features per core (2 heads x 64)
DH = 64
P = 128
DK = D // P         # 8 contraction chunks for projections
N_CORES = 8
NEG = -1e6

_CACHE: dict = {}


def _build_program(KC: tuple[int, int], cfg: dict):
    import concourse.bass as bass
    import concourse.tile as tile
    from concourse import mybir
    from concourse.masks import make_identity

    dt = mybir.dt
    DT_IN = getattr(dt, cfg["dt_in"])        # xT + W in HBM / matmul dtype
    DT_ATTN = getattr(dt, cfg["dt_attn"])    # attn / V / QT / KT storage
    DT_OUT = getattr(dt, cfg["dt_out"])      # partial output in HBM

    nc = bass.Bass("TRN2")

    xtq_d = nc.dram_tensor("xtq", [D, NT], DT_IN, kind="ExternalInput")
    xtk_d = nc.dram_tensor("xtk", [D, NT], DT_IN, kind="ExternalInput")
    xtv_d = nc.dram_tensor("xtv", [D, NT], DT_IN, kind="ExternalInput")
    wq_d = nc.dram_tensor("wq", [D, F], DT_IN, kind="ExternalInput")
    wk_d = nc.dram_tensor("wk", [D, F], DT_IN, kind="ExternalInput")
    wv_d = nc.dram_tensor("wv", [D, F], DT_IN, kind="ExternalInput")
    wo_d = nc.dram_tensor("wo", [F, D], DT_IN, kind="ExternalInput")
    mask_d = nc.dram_tensor("maskt", [P, B * 16], dt.float32, kind="ExternalInput")
    out_d = nc.dram_tensor("out_part", [NT, D], DT_OUT, kind="ExternalOutput")

    # global 512-wide seq tiles needed for K/V per batch (only up to valid_len)
    ntk = []
    for b in range(B):
        for t in range(math.ceil(KC[b] * 128 / 512)):
            ntk.append(b * 4 + t)
    NQT = NT // 512  # 8 q tiles of 512

    from contextlib import ExitStack

    B_X = int(cfg.get("b_x", 3))        # x streaming tiles per kc tag
    B_AT = int(cfg.get("b_at", 4))      # attn tiles
    B_SC = int(cfg.get("b_sc", 2))      # score psum banks
    B_AV = int(cfg.get("b_av", 2))      # av psum banks
    B_PROJ = int(cfg.get("b_proj", 1))  # proj psum banks
    B_PW = int(cfg.get("b_pw", 1))      # Wo psum banks
    assert 2 * B_SC + B_AV + B_PROJ + B_PW <= 8

    with tile.TileContext(nc) as tc, ExitStack() as ctx:
        const = ctx.enter_context(tc.tile_pool(name="const", bufs=1))
        xpool = ctx.enter_context(tc.tile_pool(name="xpool", bufs=B_X))
        apool = ctx.enter_context(tc.tile_pool(name="apool", bufs=B_AT))
        rpool = ctx.enter_context(tc.tile_pool(name="rpool", bufs=2))
        ps_score = ctx.enter_context(
            tc.tile_pool(name="ps_score", bufs=B_SC, space="PSUM"))
        ps_av = ctx.enter_context(
            tc.tile_pool(name="ps_av", bufs=B_AV, space="PSUM"))
        ps_misc = ctx.enter_context(
            tc.tile_pool(name="ps_misc", bufs=2, space="PSUM"))

        # ---- constants ----
        # Matmult instructions tolerate only ONE sync-wait, so every tensor a
        # matmul reads must be written by DVE (one mergeable semaphore): all
        # weight/identity loads bounce DRAM -> raw tile -> DVE copy -> tile.
        def dve_load(dst, src_ap, raw_shape, raw_dtype, nm):
            raw = const.tile(list(raw_shape), raw_dtype, tag=f"{nm}_raw",
                             name=f"{nm}_raw")
            nc.sync.dma_start(raw, src_ap)
            nc.vector.tensor_copy(out=dst, in_=raw)

        wq_sb = const.tile([P, DK, F], DT_IN, tag="wq")
        wk_sb = const.tile([P, DK, F], DT_IN, tag="wk")
        wv_sb = const.tile([P, DK, F], DT_IN, tag="wv")
        dve_load(wq_sb, wq_d.rearrange("(kc p) f -> p kc f", p=P),
                 [P, DK, F], DT_IN, "wq")
        dve_load(wk_sb, wk_d.rearrange("(kc p) f -> p kc f", p=P),
                 [P, DK, F], DT_IN, "wk")
        dve_load(wv_sb, wv_d.rearrange("(kc p) f -> p kc f", p=P),
                 [P, DK, F], DT_IN, "wv")
        # Wo rows split per head so contraction runs as 2 accumulating K=64
        # matmuls with partition-0-based operands (lane alignment).
        wo0_sb = const.tile([DH, D], DT_IN, tag="wo0")
        wo1_sb = const.tile([DH, D], DT_IN, tag="wo1")
        dve_load(wo0_sb, wo_d[0:DH, :], [DH, D], DT_IN, "wo0")
        dve_load(wo1_sb, wo_d[DH:2 * DH, :], [DH, D], DT_IN, "wo1")
        # mask feeds ACT (exp bias): route through an ACT copy so exp's dep
        # is ACT-program-order
        mask_raw = const.tile([P, B * 16], dt.float32, tag="mask_raw")
        nc.sync.dma_start(mask_raw, mask_d[:, :])
        mask_sb = const.tile([P, B * 16], dt.float32, tag="mask")
        nc.scalar.copy(out=mask_sb, in_=mask_raw)
        ones_sb = const.tile([P, DH], dt.float32, tag="ones")
        nc.vector.memset(ones_sb, 1.0)
        ident_g = const.tile([P, P], DT_ATTN, tag="ident_g")
        make_identity(nc, ident_g)
        ident = const.tile([P, P], DT_ATTN, tag="ident")
        nc.vector.tensor_copy(out=ident, in_=ident_g)
        VT = const.tile([P, NT], DT_ATTN, tag="VT")

        QT = const.tile([P, NT], DT_ATTN, tag="QT")
        KT = const.tile([P, NT], DT_ATTN, tag="KT")
        # V natural layout per 128-k chunk, heads split with a ones column each:
        # cols 0:64 = head0, col 64 = ones, cols 65:129 = head1, col 129 = ones
        V = const.tile([P, B * 16, 130], DT_ATTN, tag="V")
        nc.vector.memset(V[:, :, 64:65], 1.0)
        nc.vector.memset(V[:, :, 129:130], 1.0)
        # attn output (transposed, pre-Wo), one tile per head, partitions 0-63
        outT0 = const.tile([DH, NT], DT_ATTN, tag="outT0")
        outT1 = const.tile([DH, NT], DT_ATTN, tag="outT1")
        # final output staging: written once per region (no slot recycling)
        out_stage = const.tile([P, NT // P, D], DT_OUT, tag="out_stage")

        # ---- stage A: projections ----
        # kc=0's x tile flows through a DVE copy so the group-opening matmul's
        # two deps (fresh x data + psum slot recycle) merge into one DVE wait;
        # kc>0 matmuls wait only on their own x DMA lane.
        def proj(xt_d, w_sb, nts, dest):
            # nts: 1 or 2 consecutive 512-wide tiles sharing one DMA per kc
            # (256KB transfers hit much better DMA efficiency than 128KB)
            xt_r = xt_d.rearrange("(kc p) n -> p kc n", p=P)
            w = 512 * len(nts)
            n0 = nts[0] * 512
            pss = [ps_misc.tile([P, 512], dt.float32, tag="proj", bufs=B_PROJ,
                                name="ps_proj") for _ in nts]
            for kc in range(DK):
                xt = xpool.tile([P, w], DT_IN, tag=f"xt{kc}", name="xt")
                nc.sync.dma_start(xt, xt_r[:, kc, n0:n0 + w])
                if kc == 0:
                    xtc = xpool.tile([P, w], DT_IN, tag="xt0c", name="xtc")
                    nc.vector.tensor_copy(out=xtc, in_=xt)
                    xt = xtc
                for i in range(len(nts)):
                    nc.tensor.matmul(pss[i], lhsT=w_sb[:, kc, :],
                                     rhs=xt[:, i * 512:(i + 1) * 512],
                                     start=(kc == 0), stop=(kc == DK - 1))
            for i, nt in enumerate(nts):
                nc.vector.tensor_copy(out=dest[:, nt * 512:(nt + 1) * 512],
                                      in_=pss[i])

        # ---- per batch: projections then attention, so batch b+1's
        # projections (DMA/PE) overlap batch b's attention (ACT-heavy) ----
        def pairs(lst):
            return [lst[i:i + 2] for i in range(0, len(lst), 2)]

        for b in range(B):
            b_nts = [nt for nt in ntk if nt // 4 == b]
            for pr in pairs(b_nts):
                proj(xtk_d, wk_sb, pr, KT)
            for pr in pairs(b_nts):
                proj(xtv_d, wv_sb, pr, VT)
            # V natural (k on partitions) via PE transposes of VT chunks
            for kcl in range(KC[b]):
                g = b * 16 + kcl
                pst = ps_score.tile([P, P], DT_ATTN, tag="sc", name="pst")
                nc.tensor.transpose(pst, VT[:, g * 128:(g + 1) * 128], ident)
                nc.vector.tensor_copy(out=V[:, g, 0:64], in_=pst[:, 0:64])
                nc.vector.tensor_copy(out=V[:, g, 65:129], in_=pst[:, 64:128])
            for pr in pairs(list(range(4 * b, 4 * b + 4))):
                proj(xtq_d, wq_sb, pr, QT)
            for qt in range(4):
                q0 = b * S + qt * 512
                av0 = ps_av.tile([P, 512], dt.float32, tag="av", name="av0")
                av1 = ps_av.tile([P, 512], dt.float32, tag="av", name="av1")
                # zero-init on ACT (instead of start=True) so the group's
                # first matmul needs only the ACT wait; has_written stays
                # clear for PE so start=False accumulates correctly.
                nc.scalar.memzero(av0[0:65])
                nc.scalar.memzero(av1[0:65])
                for kcl in range(KC[b]):
                    g = b * 16 + kcl
                    k0 = b * S + kcl * 128
                    sc2 = ps_score.tile([P, 2, 512], dt.float32, tag="sc",
                                        name="sc2")
                    nc.tensor.matmul(sc2[:, 0, :], lhsT=KT[0:64, k0:k0 + 128],
                                     rhs=QT[0:64, q0:q0 + 512])
                    nc.tensor.matmul(sc2[:, 1, :], lhsT=KT[64:128, k0:k0 + 128],
                                     rhs=QT[64:128, q0:q0 + 512])
                    at = apool.tile([P, 2, 512], DT_ATTN, tag="at", name="at")
                    bias = mask_sb[:, g:g + 1]
                    nc.scalar.activation(at.rearrange("p a n -> p (a n)"),
                                         sc2.rearrange("p a n -> p (a n)"),
                                         mybir.ActivationFunctionType.Exp,
                                         bias=bias, scale=0.125)
                    sp = (kcl == KC[b] - 1)
                    nc.tensor.matmul(av0[0:65], lhsT=V[:, g, 0:65],
                                     rhs=at[:, 0, :], start=False, stop=sp)
                    nc.tensor.matmul(av1[0:65], lhsT=V[:, g, 65:130],
                                     rhs=at[:, 1, :], start=False, stop=sp)
                # normalize both heads: rowsum sits in row 64 of each av
                # psum. ln(rowsum) -> broadcast over 64 partitions via a K=1
                # fp32 PE matmul -> exp(-x) gives 1/rowsum on all partitions.
                for h, av, oT in ((0, av0, outT0), (1, av1, outT1)):
                    lnr = rpool.tile([65, 512], dt.float32, tag="lnr",
                                     name="lnr")
                    nc.scalar.activation(lnr[64:65, :], av[64:65, :],
                                         mybir.ActivationFunctionType.Ln)
                    bc = ps_misc.tile([P, 512], dt.float32, tag="pw",
                                      bufs=B_PW, name="bc")
                    nc.tensor.matmul(bc[0:64], lhsT=ones_sb[64:65, :],
                                     rhs=lnr[64:65, :])
                    rbc = rpool.tile([DH, 512], dt.float32, tag="rbc",
                                     name="rbc")
                    nc.scalar.activation(rbc, bc[0:64],
                                         mybir.ActivationFunctionType.Exp,
                                         scale=-1.0)
                    nc.vector.tensor_mul(out=oT[:, q0:q0 + 512],
                                         in0=av[0:64], in1=rbc)
                # Wo for this q tile: 4 s-chunks of 128, contraction split
                # into the two heads' K=64 halves (accumulated in PSUM)
                for sc in range(4):
                    gsc = (q0 + sc * 128) // 128   # global 128-row chunk
                    r0 = gsc * 128
                    for half in range(2):
                        pw = ps_misc.tile([P, 512], dt.float32, tag="pw",
                                          bufs=B_PW, name="pw")
                        w_sl = slice(half * 512, (half + 1) * 512)
                        nc.tensor.matmul(pw, lhsT=outT0[:, r0:r0 + 128],
                                         rhs=wo0_sb[:, w_sl],
                                         start=True, stop=False)
                        nc.tensor.matmul(pw, lhsT=outT1[:, r0:r0 + 128],
                                         rhs=wo1_sb[:, w_sl],
                                         start=False, stop=True)
                        nc.vector.tensor_copy(out=out_stage[:, gsc, w_sl],
                                              in_=pw)
                # one 1MB DMA out per q tile
                gs0 = q0 // 128
                nc.sync.dma_start(
                    out_d.rearrange("(g p) n -> p g n", p=P)[:, gs0:gs0 + 4, :],
                    out_stage[:, gs0:gs0 + 4, :])

    _legalize_waits(nc)
    return nc


def _legalize_waits(nc):
    """This walrus build accepts at most ONE sync-wait command per
    instruction, while Tile emits up to a dozen (e.g. the kernel-tail
    drain). Legalize by splitting: excess waits are hoisted onto
    same-engine Drain instructions inserted immediately before the
    offender — same-engine program order makes this semantically
    identical. Patched module is served via nc.to_json_bytes."""
    import json as _json

    raw = nc.to_json_bytes()
    d = _json.loads(raw)
    template = None
    for fn in d.get("functions", []):
        for blk in fn.get("blocks", []):
            for inst in blk.get("instructions", []):
                if inst.get("opcode") == "Drain":
                    template = inst
                    break
            if template:
                break
        if template:
            break
    assert template is not None, "no Drain template found"

    counter = [0]

    def carrier(engine, wait):
        counter[0] += 1
        c = _json.loads(_json.dumps(template))
        c["name"] = f"I-waitfix-{counter[0]}"
        c["engine"] = engine
        c["sync_info"] = {"on_update": [], "on_wait": [wait]}
        c["ins"] = []
        c["outs"] = []
        return c

    nfix = 0
    for fn in d.get("functions", []):
        for blk in fn.get("blocks", []):
            out = []
            for inst in blk.get("instructions", []):
                si = inst.get("sync_info")
                waits = (si or {}).get("on_wait") or []
                if len(waits) > 1:
                    for w in waits[:-1]:
                        out.append(carrier(inst["engine"], w))
                    si["on_wait"] = [waits[-1]]
                    nfix += 1
                out.append(inst)
            blk["instructions"] = out

    patched = _json.dumps(d).encode()
    nc.to_json_bytes = lambda: patched


def _fix_sync_waits(nc):
    """Walrus rejects instructions with more sync-wait commands than their
    ISA encoding holds (Matmult/Ldweights/DMACopy: 1). Tile's sem assignment
    is not transitively minimal and sometimes exceeds this. Two safe
    reductions, applied in order:

    1. Deletion by implication: drop wait W2 if a kept wait W1's producer
       instruction itself (originally) waited on the same semaphore at >= the
       required value — W1 then transitively implies W2.
    2. Relocation: move a wait onto an earlier instruction of the same
       engine (in-order, so waiting earlier is correct), positioned after
       the wait's producer (so it cannot deadlock)."""
    import bisect

    LIMITS = {"Matmult": 1, "Ldweights": 1, "DMACopy": 1}
    for f in nc.m.functions:
        for blk in f.blocks:
            insts = blk.instructions
            sem_vals: dict = {}
            sem_idx: dict = {}
            cum: dict = {}
            eng_of = []
            orig_waits = []
            for idx, inst in enumerate(insts):
                eng_of.append(str(inst.engine))
                si = inst.sync_info
                ws = []
                if si is not None:
                    ws = [(w.ant_name, w.wait_value) for w in (si.on_wait or [])]
                    for u in (si.on_update or []):
                        nm = u.ant_name
                        cum[nm] = cum.get(nm, 0) + (u.update_value or 1)
                        sem_vals.setdefault(nm, []).append(cum[nm])
                        sem_idx.setdefault(nm, []).append(idx)
                orig_waits.append(ws)

            def producer(sem, val):
                vals = sem_vals.get(sem, [])
                i = bisect.bisect_left(vals, val)
                return sem_idx[sem][i] if i < len(vals) else None

            def implied(w, kept):
                # does some kept wait's producer transitively cover w?
                seen = set()
                frontier = [(k.ant_name, k.wait_value) for k in kept]
                depth = 0
                while frontier and depth < 4:
                    nxt = []
                    for sem, val in frontier:
                        p = producer(sem, val)
                        if p is None or p in seen:
                            continue
                        seen.add(p)
                        for (s2, v2) in orig_waits[p]:
                            if s2 == w.ant_name and v2 >= w.wait_value:
                                return True
                            nxt.append((s2, v2))
                    frontier = nxt
                    depth += 1
                return False

            for idx, inst in enumerate(insts):
                if inst.opcode == "Drain":
                    continue
                si = inst.sync_info
                if si is None or not si.on_wait:
                    continue
                waits = list(si.on_wait)
                limit = LIMITS.get(inst.opcode, 2)
                if len(waits) <= limit:
                    continue
                eng = eng_of[idx]
                # keep cross-engine waits first (data deps), shed self/WAW
                self_w = [w for w in waits
                          if eng.endswith(w.ant_name.split("_")[0])]
                other_w = [w for w in waits if w not in self_w]
                ordered = other_w + self_w
                keep = ordered[:limit]
                excess = ordered[limit:]
                # try implication-deletion of excess (and also try swapping:
                # maybe a kept one is implied by an excess one)
                remaining = []
                for w in excess:
                    if implied(w, keep):
                        continue
                    swapped = False
                    for i, k in enumerate(keep):
                        trial = keep[:i] + [w] + keep[i + 1:]
                        if implied(k, trial):
                            keep = trial
                            swapped = True
                            break
                    if not swapped:
                        remaining.append(w)
                for w in remaining:
                    pidx = producer(w.ant_name, w.wait_value)
                    host = None
                    j = idx - 1
                    while j >= 0 and j > (pidx if pidx is not None else -1):
                        if eng_of[j] == eng and insts[j].opcode != "Drain":
                            hsi = insts[j].sync_info
                            hw = list(hsi.on_wait) if (
                                hsi is not None and hsi.on_wait) else []
                            hlim = LIMITS.get(insts[j].opcode, 2)
                            if hsi is not None and len(hw) < hlim and not any(
                                    x.ant_name == w.ant_name for x in hw):
                                host = (j, hsi, hw)
                                break
                        j -= 1
                    if host is None:
                        raise RuntimeError(
                            f"_fix_sync_waits: no host for {inst.name} "
                            f"wait {w.ant_name}>={w.wait_value}")
                    _, hsi, hw = host
                    hsi.on_wait = hw + [w]
                si.on_wait = keep


def _prep_host(queries, keys, values, Wq, Wk, Wv, Wo, valid_lens, cfg):
    np_in = {"bfloat16": ml_dtypes.bfloat16, "float32": np.float32,
             "float32r": np.float32, "float16": np.float16}[cfg["dt_in"]]
    L = [int(valid_lens[0]), int(valid_lens[1])]
    KC = tuple(min(16, (l + 127) // 128) for l in L)

    def t2(x):  # (B,S,D) -> (D, B*S)
        return np.ascontiguousarray(
            np.asarray(x, np.float32).reshape(NT, D).T).astype(np_in)

    xtq, xtk, xtv = t2(queries), t2(keys), t2(values)
    maskt = np.full((P, B * 16), NEG, np.float32)
    for b in range(B):
        for c in range(16):
            ks = c * 128 + np.arange(P)
            maskt[:, b * 16 + c] = np.where(ks < L[b], 0.0, NEG)

    Wq = np.asarray(Wq, np.float32)
    Wk = np.asarray(Wk, np.float32)
    Wv = np.asarray(Wv, np.float32)
    Wo = np.asarray(Wo, np.float32)
    in_maps = []
    for c in range(N_CORES):
        cs = slice(c * F, (c + 1) * F)
        in_maps.append({
            "xtq": xtq, "xtk": xtk, "xtv": xtv,
            "wq": np.ascontiguousarray(Wq[:, cs]).astype(np_in),
            "wk": np.ascontiguousarray(Wk[:, cs]).astype(np_in),
            "wv": np.ascontiguousarray(Wv[:, cs]).astype(np_in),
            "wo": np.ascontiguousarray(Wo[cs, :]).astype(np_in),
            "maskt": maskt,
        })
    return KC, in_maps


DEFAULT_CFG = {"dt_in": "float16", "dt_attn": "float16", "dt_out": "float16"}

LAST_RESULTS = None


def kernel(queries, keys, values, Wq, Wk, Wv, Wo, valid_lens):
    global LAST_RESULTS
    from concourse.bass_utils import run_bass_kernel_spmd

    cfg = dict(DEFAULT_CFG)
    if os.environ.get("MHA_CFG"):
        for kv in os.environ["MHA_CFG"].split(","):
            k, v = kv.split("=")
            cfg[k] = v

    KC, in_maps = _prep_host(queries, keys, values, Wq, Wk, Wv, Wo,
                             valid_lens, cfg)
    key = (KC, tuple(sorted(cfg.items())))
    if key not in _CACHE:
        _CACHE[key] = _build_program(KC, cfg)
    nc = _CACHE[key]

    trace = bool(os.environ.get("MHA_TRACE"))
    res = run_bass_kernel_spmd(nc, in_maps, core_ids=list(range(N_CORES)),
                               trace=trace)
    LAST_RESULTS = res
    acc = np.zeros((NT, D), np.float32)
    for r in res.results:
        acc += np.asarray(r["out_part"], np.float32)
    return acc.reshape(B, S, D)



# revision 22
# speedup vs baseline: 1.2099x; 1.2099x over previous
"""Multi-head attention (16 heads, D=1024, B=2, S=2048) on 8 TRN2 NeuronCores.

Sharding: tensor-parallel over heads. Each core owns 2 heads (128 features):
W_q/k/v column-sliced, W_o row-sliced; partial outputs summed on host.

Device dataflow (per core), everything kept "transposed" (features on
partitions) so the key-padding mask folds into the ACT exp bias and the
attention matrix is produced directly in the layout the A@V matmul needs:

  QT[f,s] = Wq_c^T @ x^T        (PE, contraction d on partitions)
  KT[f,s] likewise; VT -> PE-transpose -> V[s,f] (natural, k on partitions)
  scores^T[k,q] = KT_h^T-slice . QT_h  (row-tiled pairs, 2 heads)
  attn^T = exp(scores*0.125 + mask_bias[k])   (ACT, PSUM->SBUF, fp16)
  A@V packed across heads: head0 -> psum av0[0:65] via [V0|1], head1 ->
  psum av1[63:128] via [1|V1] (separate banks), so the normalized output
  lands in ONE [128, q] outT tile and W_o runs as single K=128 matmuls
  (half the PE row-streams of a split-head contraction).
  rowsums sit at av0[64] / av1[63]; 1/r via DVE reciprocal (fp16), ONE
  K=2 fp16 matmul broadcasts both heads' 1/r over the 128 partitions.
  out_part[s,:] = outT^T . Wo_c  -> fp16 partial, host sums in fp32

Emission is pipelined for the DMA-bound front: per batch, K/V tile-pairs
are projected in segments with qt0's attention chunks interleaved, so PE
works on early k-chunks while later x tiles are still in flight.

Key-padding mask: k-chunks entirely beyond valid_len are skipped (program
is specialized to the valid_lens values at call time); the boundary chunk
uses a -1e6 additive bias inside the exp activation (exp underflows to 0).
"""

import math
import os

import ml_dtypes
import numpy as np

B = 2
S = 2048
D = 1024
NT = B * S          # 4096 rows, b-major
F = 128             # features per core (2 heads x 64)
DH = 64
P = 128
DK = D // P         # 8 contraction chunks for projections
N_CORES = 8
NEG = -1e6

# byte offsets (in elements) of each weight block inside the packed wall
WQ_BASE = 0
WK_BASE = DK * F
WV_BASE = 2 * DK * F
WO_BASE = 3 * DK * F
WALL_W = 3 * DK * F + D

_CACHE: dict = {}
MM_LABELS: list = []


def _build_program(KC: tuple[int, int], cfg: dict):
    import concourse.bass as bass
    import concourse.tile as tile
    from concourse import mybir
    from concourse.masks import make_identity

    dt = mybir.dt
    DT_IN = getattr(dt, cfg["dt_in"])        # xT + W in HBM / matmul dtype
    DT_ATTN = getattr(dt, cfg["dt_attn"])    # attn / V / QT / KT storage
    DT_OUT = getattr(dt, cfg["dt_out"])      # partial output in HBM

    nc = bass.Bass("TRN2")
    MM_LABELS.clear()
    _real_mm = nc.tensor.matmul
    _real_tp = nc.tensor.transpose

    def _mm(*a, _lab=None, **k):
        MM_LABELS.append(_mm_label[0])
        return _real_mm(*a, **k)

    def _tp(*a, **k):
        return _real_tp(*a, **k)

    _mm_label = ["?"]
    nc.tensor.matmul = _mm
    nc.tensor.transpose = _tp

    def _lab(s):
        _mm_label[0] = s

    xtq_d = nc.dram_tensor("xtq", [D, NT], DT_IN, kind="ExternalInput")
    xtk_d = nc.dram_tensor("xtk", [D, NT], DT_IN, kind="ExternalInput")
    xtv_d = nc.dram_tensor("xtv", [D, NT], DT_IN, kind="ExternalInput")
    wall_d = nc.dram_tensor("wall", [P, WALL_W], DT_IN, kind="ExternalInput")
    mask_d = nc.dram_tensor("maskt", [P, B * 16], dt.float32, kind="ExternalInput")
    out_d = nc.dram_tensor("out_part", [NT, D], DT_OUT, kind="ExternalOutput")

    from contextlib import ExitStack

    B_X = int(cfg.get("b_x", 3))        # x streaming tiles per kc tag
    B_AT = int(cfg.get("b_at", 6))      # attn tiles
    B_SC = int(cfg.get("b_sc", 2))      # score psum bufs (2 banks each)
    B_AV = 2                            # av psum banks (both live per qt)
    B_PW = int(cfg.get("b_pw", 2))      # shared proj/bc/wo psum banks
    assert 2 * B_SC + B_AV + B_PW <= 8

    with tile.TileContext(nc) as tc, ExitStack() as ctx:
        const = ctx.enter_context(tc.tile_pool(name="const", bufs=1))
        xpool = ctx.enter_context(tc.tile_pool(name="xpool", bufs=B_X))
        apool = ctx.enter_context(tc.tile_pool(name="apool", bufs=B_AT))
        rpool = ctx.enter_context(tc.tile_pool(name="rpool", bufs=2))
        opool = ctx.enter_context(tc.tile_pool(name="opool", bufs=4))
        ps_score = ctx.enter_context(
            tc.tile_pool(name="ps_score", bufs=B_SC, space="PSUM"))
        ps_av = ctx.enter_context(
            tc.tile_pool(name="ps_av", bufs=B_AV, space="PSUM"))
        ps_pw = ctx.enter_context(
            tc.tile_pool(name="ps_pw", bufs=B_PW, space="PSUM"))

        # ---- constants ----
        # Matmult instructions tolerate only ONE sync-wait, so tensors a
        # matmul reads are written by DVE (one mergeable semaphore): the
        # packed weight wall bounces DRAM -> raw tile -> DVE copy -> tile.
        wall_raw = const.tile([P, WALL_W], DT_IN, tag="wall_raw")
        wall = const.tile([P, WALL_W], DT_IN, tag="wall")
        _wsec_done = set()

        def w_sec(base, width=DK * F):
            # DMA+copy one weight section, emitted lazily right before its
            # first consumer so the DMA queue stays in true dependency order
            if base in _wsec_done:
                return
            _wsec_done.add(base)
            nc.sync.dma_start(wall_raw[:, base:base + width],
                              wall_d[:, base:base + width])
            nc.vector.tensor_copy(out=wall[:, base:base + width],
                                  in_=wall_raw[:, base:base + width])

        mask_raw = const.tile([P, B * 16], dt.float32, tag="mask_raw")
        mask_sb = const.tile([P, B * 16], dt.float32, tag="mask")

        ident_g = const.tile([P, P], DT_ATTN, tag="ident_g")
        make_identity(nc, ident_g)
        ident = const.tile([P, P], DT_ATTN, tag="ident")
        nc.vector.tensor_copy(out=ident, in_=ident_g)

        # ones row at p64 (where both rowsums live) for the K=1 1/r
        # broadcast matmuls
        onesk = const.tile([P, DH], DT_ATTN, tag="onesk")
        nc.vector.memset(onesk[64:65, :], 1.0)

        QT = const.tile([P, NT], DT_ATTN, tag="QT")
        KT = const.tile([P, NT], DT_ATTN, tag="KT")
        VT = const.tile([P, NT], DT_ATTN, tag="VT")
        # V natural layout per 128-k chunk, heads split with a ones column
        # each: [V0(0:64) | 1(64) | V1(65:129) | 1(129)]
        V = const.tile([P, B * 16, 130], DT_ATTN, tag="V")
        nc.vector.memset(V[:, :, 64:65], 1.0)
        nc.vector.memset(V[:, :, 129:130], 1.0)
        # attn output (transposed, pre-Wo): head0 on p0:64, head1 on p64:128
        outT = const.tile([P, NT], DT_ATTN, tag="outT")

        # ---- stage A: projections ----
        # kc=0's x tile flows through a DVE copy so the group-opening matmul's
        # two deps (fresh x data + psum slot recycle) merge into one DVE wait;
        # kc>0 matmuls wait only on their own x DMA lane.
        def proj(xt_d, wbase, nts, dest, granular=False, n0=None, w=None):
            _lab(f"proj_{'qkv'[[WQ_BASE, WK_BASE, WV_BASE].index(wbase)] if wbase != WO_BASE else 'o'}")
            # nts: 1 or 2 consecutive 512-wide tiles; x arrives as two
            # 4-chunk batched DMAs (HWDGE issue cost ~625ns/op dominates
            # many-small-DMA schedules; descriptor count is unchanged).
            # n0/w override the token range (trim K/V past valid_len).
            xt_r = xt_d.rearrange("(kc p) n -> p kc n", p=P)
            if w is None:
                w = 512 * len(nts)
            if n0 is None:
                n0 = nts[0] * 512
            pss = [ps_pw.tile([P, 512], dt.float32, tag="pw", name="ps_proj")
                   for _ in nts]
            HK = DK // 2
            xts = []
            for g in range(2):
                xt = xpool.tile([P, HK, w], DT_IN, tag=f"xt{g}", name="xt")
                if granular:
                    for kc in range(HK):
                        nc.sync.dma_start(xt[:, kc:kc + 1, :],
                                          xt_r[:, g * HK + kc:g * HK + kc + 1,
                                               n0:n0 + w])
                else:
                    nc.sync.dma_start(xt, xt_r[:, g * HK:(g + 1) * HK,
                                               n0:n0 + w])
                xts.append(xt)
            for kc in range(DK):
                xt = xts[kc // HK][:, kc % HK, :]
                for i in range(len(nts)):
                    wi = min(512, w - i * 512)
                    nc.tensor.matmul(
                        pss[i][:, 0:wi],
                        lhsT=wall[:, wbase + kc * F:wbase + (kc + 1) * F],
                        rhs=xt[:, i * 512:i * 512 + wi],
                        start=(kc == 0), stop=(kc == DK - 1))
            for i, nt in enumerate(nts):
                wi = min(512, w - i * 512)
                nc.vector.tensor_copy(out=dest[:, n0 + i * 512:n0 + i * 512 + wi],
                                      in_=pss[i][:, 0:wi])

        def pairs(lst):
            return [lst[i:i + 2] for i in range(0, len(lst), 2)]

        def v_nat(b, kcl):
            _lab(f"vT_{b}")
            # V natural (k on partitions) via PE transpose of a VT chunk
            g = b * 16 + kcl
            pst = ps_score.tile([P, P], DT_ATTN, tag="sc", name="pst")
            nc.tensor.transpose(pst, VT[:, g * 128:(g + 1) * 128], ident)
            nc.vector.tensor_copy(out=V[:, g, 0:64], in_=pst[:, 0:64])
            nc.vector.tensor_copy(out=V[:, g, 65:129], in_=pst[:, 64:128])

        def scores_exp(b, qt, kcl):
            # scores + exp for one k-chunk; returns the at tile. Emitted
            # ahead of the previous qt's normalize chain, this is PE/ACT
            # work with no dependence on the av accumulators.
            _lab(f"sc_{b}")
            q0 = b * S + qt * 512
            g = b * 16 + kcl
            k0 = b * S + kcl * 128
            sc2 = ps_score.tile([P, 2, 512], dt.float32, tag="sc", name="sc2")
            nc.tensor.matmul(sc2[:, 0, :], lhsT=KT[0:64, k0:k0 + 128],
                             rhs=QT[0:64, q0:q0 + 512])
            nc.tensor.matmul(sc2[:, 1, :], lhsT=KT[64:128, k0:k0 + 128],
                             rhs=QT[64:128, q0:q0 + 512])
            at = apool.tile([P, 2, 512], DT_ATTN, tag="at", name="at")
            bias = mask_sb[:, g:g + 1]
            nc.scalar.activation(at.rearrange("p a n -> p (a n)"),
                                 sc2.rearrange("p a n -> p (a n)"),
                                 mybir.ActivationFunctionType.Exp,
                                 bias=bias, scale=0.125)
            return at

        def av_mms(b, av0, av1, kcl, at):
            _lab(f"av_{b}")
            g = b * 16 + kcl
            sp = (kcl == KC[b] - 1)
            nc.tensor.matmul(av0[0:65], lhsT=V[:, g, 0:65],
                             rhs=at[:, 0, :], start=(kcl == 0), stop=sp)
            nc.tensor.matmul(av1[0:65], lhsT=V[:, g, 65:130],
                             rhs=at[:, 1, :], start=(kcl == 0), stop=sp)

        def attn_chunks(b, qt, av0, av1, kcls, pre=()):
            for i, kcl in enumerate(kcls):
                at = pre[i] if i < len(pre) else scores_exp(b, qt, kcl)
                av_mms(b, av0, av1, kcl, at)

        def avc_copies(av0, av1):
            # drain the av accumulators to SBUF right after the last AV
            # matmul: frees the 2 av PSUM banks for the next q tile and
            # lets the whole normalize chain run deferred (SBUF x PSUM
            # TensorTensor is legal; PSUM x PSUM is not)
            avc0 = rpool.tile([65, 512], DT_ATTN, tag="avc0", name="avc0")
            avc1 = rpool.tile([65, 512], DT_ATTN, tag="avc1", name="avc1")
            with nc.allow_low_precision(reason="attn out fp16 everywhere"):
                nc.vector.tensor_copy(out=avc0, in_=av0[0:65])
                nc.vector.tensor_copy(out=avc1, in_=av1[0:65])
            return avc0, avc1

        def norm_qt(b, qt, avc0, avc1):
            _lab(f"nrm_{b}")
            q0 = b * S + qt * 512
            # normalize both heads: rowsums at avc0[64] / avc1[64]. 1/r on
            # DVE (fp16, values <= 1), K=1 fp16 matmuls broadcast each 1/r
            # over 64 partitions; DVE multiplies SBUF x PSUM. head1's
            # normalized tile is shifted to partitions 64:128 with a K=64
            # identity matmul (PE is the only partition mover) so Wo
            # contracts both heads in single K=128 matmuls.
            rinv = rpool.tile([P, 2, 512], DT_ATTN, tag="rinv", name="rinv")
            with nc.allow_low_precision(reason="1/rowsum <= 1 fits fp16"):
                nc.vector.reciprocal(out=rinv[64:65, 0, :],
                                     in_=avc0[64:65, :])
                nc.vector.reciprocal(out=rinv[64:65, 1, :],
                                     in_=avc1[64:65, :])
            bc0 = ps_pw.tile([P, 512], dt.float32, tag="pw", name="bc0")
            nc.tensor.matmul(bc0[0:64], lhsT=onesk[64:65, :],
                             rhs=rinv[64:65, 0, :])
            nc.vector.tensor_mul(out=outT[0:64, q0:q0 + 512],
                                 in0=avc0[0:64], in1=bc0[0:64])
            bc1 = ps_pw.tile([P, 512], dt.float32, tag="pw", name="bc1")
            nc.tensor.matmul(bc1[0:64], lhsT=onesk[64:65, :],
                             rhs=rinv[64:65, 1, :])
            tmp1 = rpool.tile([DH, 512], DT_ATTN, tag="tmp1", name="tmp1")
            nc.vector.tensor_mul(out=tmp1, in0=avc1[0:64], in1=bc1[0:64])
            mv = ps_pw.tile([P, 512], dt.float32, tag="pw", name="mv")
            nc.tensor.matmul(mv[64:128], lhsT=ident[0:64, 0:64], rhs=tmp1)
            nc.vector.tensor_copy(out=outT[64:128, q0:q0 + 512],
                                  in_=mv[64:128])

        def wo_units(b, qt, last=False):
            # Wo for one q tile as 4 deferred closures (one 128-row s-chunk
            # each: 2 K=128 matmuls + DVE drains + out DMA every 2 chunks).
            # Interleaved into the NEXT q tile's attention so the PSUM
            # drains overlap exp-paced slack instead of stalling PE.
            q0 = b * S + qt * 512
            gs0 = q0 // 128
            ost = opool.tile([P, 4, D], DT_OUT, tag="ost", name="ost")

            def unit(sci):
                def emit():
                    _lab(f"wo_{b}")
                    r0 = (gs0 + sci) * 128
                    for half in range(2):
                        pw = ps_pw.tile([P, 512], dt.float32, tag="pw",
                                        name="pw")
                        w_sl = slice(half * 512, (half + 1) * 512)
                        nc.tensor.matmul(pw, lhsT=outT[:, r0:r0 + 128],
                                         rhs=wall[:, WO_BASE + w_sl.start:
                                                  WO_BASE + w_sl.stop])
                        if last and half == 1:
                            nc.scalar.copy(out=ost[:, sci, w_sl], in_=pw)
                        else:
                            nc.vector.tensor_copy(out=ost[:, sci, w_sl],
                                                  in_=pw)
                    if last:
                        nc.sync.dma_start(
                            out_d.rearrange(
                                "(g p) n -> p g n",
                                p=P)[:, gs0 + sci:gs0 + sci + 1, :],
                            ost[:, sci:sci + 1, :])
                    elif sci == 1 or sci == 3:
                        nc.sync.dma_start(
                            out_d.rearrange(
                                "(g p) n -> p g n",
                                p=P)[:, gs0 + sci - 1:gs0 + sci + 1, :],
                            ost[:, sci - 1:sci + 1, :])
                return emit
            return [unit(s) for s in range(4)]

        # ---- per batch: segmented K/V projection with qt0's attention
        # chunks interleaved, then qt1..3 streaming; Wo work of each qt is
        # deferred into the next qt's attention loop ----
        wo_todo = []

        def attn_seq(b, qt, av0, av1, kcls, i0=0, pre=()):
            for i, kcl in enumerate(kcls):
                at = pre[i] if i < len(pre) else scores_exp(b, qt, kcl)
                av_mms(b, av0, av1, kcl, at)
                if i0 + i >= 2 and wo_todo:
                    wo_todo.pop(0)()

        pending = None  # (b, qt, av0, av1) whose normalize awaits cover

        for b in range(B):
            n_kv_tiles = math.ceil(KC[b] * 128 / 512)
            av0 = ps_av.tile([P, 512], dt.float32, tag="av", name="av0")
            av1 = ps_av.tile([P, 512], dt.float32, tag="av", name="av1")
            for si in range(n_kv_tiles):
                t = b * 4 + si
                g = (si == 0)
                n0 = b * S + si * 512
                wkv = min(512, KC[b] * 128 - si * 512)
                c_lo = si * 4
                c_hi = min(c_lo + 4, KC[b])
                pre0 = ()
                w_sec(WK_BASE)
                proj(xtk_d, WK_BASE, [t], KT, granular=g, n0=n0, w=wkv)
                if si == 0:
                    # Q + scores before V: scores need only K and Q, so PE
                    # streams them while V's x tiles are still in flight
                    w_sec(WQ_BASE)
                    proj(xtq_d, WQ_BASE, [b * 4], QT, granular=g)
                    if b == 0:
                        nc.sync.dma_start(mask_raw, mask_d[:, :])
                        nc.scalar.copy(out=mask_sb, in_=mask_raw)
                    if pending is not None:
                        # b's first segment covers the previous batch's
                        # last normalize/Wo chain
                        pn = pending
                        wo_todo.append(lambda pn=pn: norm_qt(*pn))
                        wo_todo.extend(wo_units(pn[0], pn[1]))
                        pending = None
                    pre0 = [scores_exp(b, 0, kcl)
                            for kcl in range(c_lo, c_hi)]
                w_sec(WV_BASE)
                proj(xtv_d, WV_BASE, [t], VT, granular=g, n0=n0, w=wkv)
                for kcl in range(c_lo, c_hi):
                    v_nat(b, kcl)
                attn_seq(b, 0, av0, av1, list(range(c_lo, c_hi)), i0=c_lo,
                         pre=pre0)
            for qt in range(3):
                proj(xtq_d, WQ_BASE, [b * 4 + qt + 1], QT)
                if qt == 0:
                    w_sec(WO_BASE, D)
                c0, c1 = avc_copies(av0, av1)
                wo_todo.append(lambda c0=c0, c1=c1, b=b, qt=qt:
                               norm_qt(b, qt, c0, c1))
                wo_todo.extend(wo_units(b, qt))
                av0 = ps_av.tile([P, 512], dt.float32, tag="av", name="av0")
                av1 = ps_av.tile([P, 512], dt.float32, tag="av", name="av1")
                attn_seq(b, qt + 1, av0, av1, list(range(KC[b])))
            for u in wo_todo:
                u()
            wo_todo = []
            c0, c1 = avc_copies(av0, av1)
            pending = (b, 3, c0, c1)
        if pending is not None:
            norm_qt(*pending)
            for u in wo_units(pending[0], pending[1], last=True):
                u()

    _legalize_waits(nc)
    return nc


def _legalize_waits(nc):
    """This walrus build accepts at most ONE sync-wait command per
    instruction, while Tile emits up to a dozen (e.g. the kernel-tail
    drain). Legalize by splitting: excess waits are hoisted onto
    same-engine Drain instructions inserted immediately before the
    offender — same-engine program order makes this semantically
    identical. Patched module is served via nc.to_json_bytes."""
    import json as _json

    raw = nc.to_json_bytes()
    d = _json.loads(raw)
    template = None
    for fn in d.get("functions", []):
        for blk in fn.get("blocks", []):
            for inst in blk.get("instructions", []):
                if inst.get("opcode") == "Drain":
                    template = inst
                    break
            if template:
                break
        if template:
            break
    assert template is not None, "no Drain template found"

    counter = [0]

    def carrier(engine, wait):
        counter[0] += 1
        c = _json.loads(_json.dumps(template))
        c["name"] = f"I-waitfix-{counter[0]}"
        c["engine"] = engine
        c["sync_info"] = {"on_update": [], "on_wait": [wait]}
        c["ins"] = []
        c["outs"] = []
        return c

    nfix = 0
    for fn in d.get("functions", []):
        for blk in fn.get("blocks", []):
            out = []
            for inst in blk.get("instructions", []):
                si = inst.get("sync_info")
                waits = (si or {}).get("on_wait") or []
                if len(waits) > 1:
                    for w in waits[:-1]:
                        out.append(carrier(inst["engine"], w))
                    si["on_wait"] = [waits[-1]]
                    nfix += 1
                out.append(inst)
            blk["instructions"] = out

    patched = _json.dumps(d).encode()
    nc.to_json_bytes = lambda: patched


def _prep_host(queries, keys, values, Wq, Wk, Wv, Wo, valid_lens, cfg):
    np_in = {"bfloat16": ml_dtypes.bfloat16, "float32": np.float32,
             "float32r": np.float32, "float16": np.float16}[cfg["dt_in"]]
    L = [int(valid_lens[0]), int(valid_lens[1])]
    KC = tuple(min(16, (l + 127) // 128) for l in L)

    def t2(x):  # (B,S,D) -> (D, B*S)
        return np.ascontiguousarray(
            np.asarray(x, np.float32).reshape(NT, D).T).astype(np_in)

    xtq, xtk, xtv = t2(queries), t2(keys), t2(values)
    maskt = np.full((P, B * 16), NEG, np.float32)
    for b in range(B):
        for c in range(16):
            ks = c * 128 + np.arange(P)
            maskt[:, b * 16 + c] = np.where(ks < L[b], 0.0, NEG)

    Wq = np.asarray(Wq, np.float32)
    Wk = np.asarray(Wk, np.float32)
    Wv = np.asarray(Wv, np.float32)
    Wo = np.asarray(Wo, np.float32)

    def packw(Wx, cs):  # [D, F] slice -> [P, DK*F], p-major rows (2KB elems)
        return Wx[:, cs].reshape(DK, P, F).transpose(1, 0, 2).reshape(P, DK * F)

    in_maps = []
    for c in range(N_CORES):
        cs = slice(c * F, (c + 1) * F)
        wall = np.concatenate(
            [packw(Wq, cs), packw(Wk, cs), packw(Wv, cs), Wo[cs, :]],
            axis=1)
        in_maps.append({
            "xtq": xtq, "xtk": xtk, "xtv": xtv,
            "wall": np.ascontiguousarray(wall).astype(np_in),
            "maskt": maskt,
        })
    return KC, in_maps


DEFAULT_CFG = {"dt_in": "float16", "dt_attn": "float16", "dt_out": "float16"}

LAST_RESULTS = None


def kernel(queries, keys, values, Wq, Wk, Wv, Wo, valid_lens):
    global LAST_RESULTS
    from concourse.bass_utils import run_bass_kernel_spmd

    cfg = dict(DEFAULT_CFG)
    if os.environ.get("MHA_CFG"):
        for kv in os.environ["MHA_CFG"].split(","):
            k, v = kv.split("=")
            cfg[k] = v

    KC, in_maps = _prep_host(queries, keys, values, Wq, Wk, Wv, Wo,
                             valid_lens, cfg)
    key = (KC, tuple(sorted(cfg.items())))
    if key not in _CACHE:
        _CACHE[key] = _build_program(KC, cfg)
    nc = _CACHE[key]

    trace = bool(os.environ.get("MHA_TRACE"))
    res = run_bass_kernel_spmd(nc, in_maps, core_ids=list(range(N_CORES)),
                               trace=trace)
    LAST_RESULTS = res
    acc = np.zeros((NT, D), np.float32)
    for r in res.results:
        acc += np.asarray(r["out_part"], np.float32)
    return acc.reshape(B, S, D)


# revision 37
# speedup vs baseline: 1.2689x; 1.0488x over previous
"""Multi-head attention (16 heads, D=1024, B=2, S=2048) on 8 TRN2 NeuronCores.

Sharding: tensor-parallel over heads. Each core owns 2 heads (128 features):
W_q/k/v column-sliced, W_o row-sliced; partial outputs summed on host.

Device dataflow (per core), everything kept "transposed" (features on
partitions) so the key-padding mask folds into the ACT exp bias and the
attention matrix is produced directly in the layout the A@V matmul needs:

  QT[f,s] = Wq_c^T @ x^T        (PE, contraction d on partitions)
  KT[f,s] likewise; VT -> PE-transpose -> V[s,f] (natural, k on partitions)
  scores^T[k,q] = KT_h^T-slice . QT_h  (row-tiled pairs, 2 heads)
  attn^T = exp(scores*0.125 + mask_bias[k])   (ACT, PSUM->SBUF, fp16)
  A@V packed across heads: head0 -> psum av0[0:65] via [V0|1], head1 ->
  psum av1[63:128] via [1|V1] (separate banks), so the normalized output
  lands in ONE [128, q] outT tile and W_o runs as single K=128 matmuls
  (half the PE row-streams of a split-head contraction).
  rowsums sit at av0[64] / av1[63]; 1/r via DVE reciprocal (fp16), ONE
  K=2 fp16 matmul broadcasts both heads' 1/r over the 128 partitions.
  out_part[s,:] = outT^T . Wo_c  -> fp16 partial, host sums in fp32

Emission is pipelined for the DMA-bound front: per batch, K/V tile-pairs
are projected in segments with qt0's attention chunks interleaved, so PE
works on early k-chunks while later x tiles are still in flight.

Key-padding mask: k-chunks entirely beyond valid_len are skipped (program
is specialized to the valid_lens values at call time); the boundary chunk
uses a -1e6 additive bias inside the exp activation (exp underflows to 0).
"""

import math
import os

import ml_dtypes
import numpy as np

B = 2
S = 2048
D = 1024
NT = B * S          # 4096 rows, b-major
F = 128             # features per core (2 heads x 64)
DH = 64
P = 128
DK = D // P         # 8 contraction chunks for projections
N_CORES = 8
NEG = -1e6

# byte offsets (in elements) of each weight block inside the packed wall
WQ_BASE = 0
WK_BASE = DK * F
WV_BASE = 2 * DK * F
WO_BASE = 3 * DK * F
WALL_W = 3 * DK * F + D

_CACHE: dict = {}
MM_LABELS: list = []


def _build_program(KC: tuple[int, int], cfg: dict):
    import concourse.bass as bass
    import concourse.tile as tile
    from concourse import mybir
    from concourse.masks import make_identity

    dt = mybir.dt
    DT_IN = getattr(dt, cfg["dt_in"])        # xT + W in HBM / matmul dtype
    DT_ATTN = getattr(dt, cfg["dt_attn"])    # attn / V / QT / KT storage
    DT_OUT = getattr(dt, cfg["dt_out"])      # partial output in HBM

    nc = bass.Bass("TRN2")
    MM_LABELS.clear()
    _real_mm = nc.tensor.matmul
    _real_tp = nc.tensor.transpose

    def _mm(*a, _lab=None, **k):
        MM_LABELS.append(_mm_label[0])
        return _real_mm(*a, **k)

    def _tp(*a, **k):
        return _real_tp(*a, **k)

    _mm_label = ["?"]
    nc.tensor.matmul = _mm
    nc.tensor.transpose = _tp

    def _lab(s):
        _mm_label[0] = s

    xtq_d = nc.dram_tensor("xtq", [D, NT], DT_IN, kind="ExternalInput")
    xtk_d = nc.dram_tensor("xtk", [D, NT], DT_IN, kind="ExternalInput")
    xtv_d = nc.dram_tensor("xtv", [D, NT], DT_IN, kind="ExternalInput")
    wall_d = nc.dram_tensor("wall", [P, WALL_W], DT_IN, kind="ExternalInput")
    mask_d = nc.dram_tensor("maskt", [P, B * 16], dt.float32, kind="ExternalInput")
    out_d = nc.dram_tensor("out_part", [NT, D], DT_OUT, kind="ExternalOutput")

    from contextlib import ExitStack

    B_X = int(cfg.get("b_x", 3))        # x streaming tiles per kc tag
    B_AT = int(cfg.get("b_at", 6))      # attn tiles
    B_SC = int(cfg.get("b_sc", 2))      # score psum bufs (2 banks each)
    B_AV = 2                            # av psum banks (both live per qt)
    B_PW = int(cfg.get("b_pw", 2))      # shared proj/bc/wo psum banks
    assert 2 * B_SC + B_AV + B_PW <= 8

    with tile.TileContext(nc) as tc, ExitStack() as ctx:
        const = ctx.enter_context(tc.tile_pool(name="const", bufs=1))
        xpool = ctx.enter_context(tc.tile_pool(name="xpool", bufs=B_X))
        apool = ctx.enter_context(tc.tile_pool(name="apool", bufs=B_AT))
        rpool = ctx.enter_context(tc.tile_pool(name="rpool", bufs=2))
        opool = ctx.enter_context(tc.tile_pool(name="opool", bufs=4))
        ps_score = ctx.enter_context(
            tc.tile_pool(name="ps_score", bufs=B_SC, space="PSUM"))
        ps_av = ctx.enter_context(
            tc.tile_pool(name="ps_av", bufs=B_AV, space="PSUM"))
        ps_pw = ctx.enter_context(
            tc.tile_pool(name="ps_pw", bufs=B_PW, space="PSUM"))

        # ---- constants ----
        # Matmult instructions tolerate only ONE sync-wait, so tensors a
        # matmul reads are written by DVE (one mergeable semaphore): the
        # packed weight wall bounces DRAM -> raw tile -> DVE copy -> tile.
        wall_raw = const.tile([P, WALL_W], DT_IN, tag="wall_raw")
        wall = const.tile([P, WALL_W], DT_IN, tag="wall")
        _wsec_done = set()

        def w_sec(base, width=DK * F):
            # DMA+copy one weight section, emitted lazily right before its
            # first consumer so the DMA queue stays in true dependency order
            if base in _wsec_done:
                return
            _wsec_done.add(base)
            nc.sync.dma_start(wall_raw[:, base:base + width],
                              wall_d[:, base:base + width])
            nc.vector.tensor_copy(out=wall[:, base:base + width],
                                  in_=wall_raw[:, base:base + width])

        mask_raw = const.tile([P, B * 16], dt.float32, tag="mask_raw")
        mask_sb = const.tile([P, B * 16], dt.float32, tag="mask")

        ident_g = const.tile([P, P], DT_ATTN, tag="ident_g")
        make_identity(nc, ident_g)
        ident = const.tile([P, P], DT_ATTN, tag="ident")
        nc.vector.tensor_copy(out=ident, in_=ident_g)

        # ones row at p64 (where both rowsums live) for the K=1 1/r
        # broadcast matmuls
        onesk = const.tile([P, DH], DT_ATTN, tag="onesk")
        nc.vector.memset(onesk[64:65, :], 1.0)

        QT = const.tile([P, NT], DT_ATTN, tag="QT")
        KT = const.tile([P, NT], DT_ATTN, tag="KT")
        VT = const.tile([P, NT], DT_ATTN, tag="VT")
        # V natural layout per 128-k chunk, heads split with a ones column
        # each: [V0(0:64) | 1(64) | V1(65:129) | 1(129)]
        V = const.tile([P, B * 16, 130], DT_ATTN, tag="V")
        nc.vector.memset(V[:, :, 64:65], 1.0)
        nc.vector.memset(V[:, :, 129:130], 1.0)
        # attn output (transposed, pre-Wo): head0 on p0:64, head1 on p64:128
        outT = const.tile([P, NT], DT_ATTN, tag="outT")

        # ---- stage A: projections ----
        # kc=0's x tile flows through a DVE copy so the group-opening matmul's
        # two deps (fresh x data + psum slot recycle) merge into one DVE wait;
        # kc>0 matmuls wait only on their own x DMA lane.
        def proj(xt_d, wbase, nts, dest, granular=False, n0=None, w=None):
            _lab(f"proj_{'qkv'[[WQ_BASE, WK_BASE, WV_BASE].index(wbase)] if wbase != WO_BASE else 'o'}")
            # nts: 1 or 2 consecutive 512-wide tiles; x arrives as two
            # 4-chunk batched DMAs (HWDGE issue cost ~625ns/op dominates
            # many-small-DMA schedules; descriptor count is unchanged).
            # n0/w override the token range (trim K/V past valid_len).
            xt_r = xt_d.rearrange("(kc p) n -> p kc n", p=P)
            if w is None:
                w = 512 * len(nts)
            if n0 is None:
                n0 = nts[0] * 512
            pss = [ps_pw.tile([P, 512], dt.float32, tag="pw", name="ps_proj")
                   for _ in nts]
            HK = DK // 2
            xts = []
            for g in range(2):
                xt = xpool.tile([P, HK, w], DT_IN, tag=f"xt{g}", name="xt")
                if granular and g == 0:
                    # two half-group DMAs for the very first call only: gets
                    # PE started earlier; 256KB transfers still hide the
                    # 625ns HWDGE issue overhead (128KB ones would not)
                    for kc in range(0, HK, 2):
                        nc.sync.dma_start(xt[:, kc:kc + 2, :],
                                          xt_r[:, g * HK + kc:g * HK + kc + 2,
                                               n0:n0 + w])
                else:
                    nc.sync.dma_start(xt, xt_r[:, g * HK:(g + 1) * HK,
                                               n0:n0 + w])
                xts.append(xt)
            for kc in range(DK):
                xt = xts[kc // HK][:, kc % HK, :]
                for i in range(len(nts)):
                    wi = min(512, w - i * 512)
                    nc.tensor.matmul(
                        pss[i][:, 0:wi],
                        lhsT=wall[:, wbase + kc * F:wbase + (kc + 1) * F],
                        rhs=xt[:, i * 512:i * 512 + wi],
                        start=(kc == 0), stop=(kc == DK - 1))
            for i, nt in enumerate(nts):
                wi = min(512, w - i * 512)
                nc.vector.tensor_copy(out=dest[:, n0 + i * 512:n0 + i * 512 + wi],
                                      in_=pss[i][:, 0:wi])

        def pairs(lst):
            return [lst[i:i + 2] for i in range(0, len(lst), 2)]

        def v_nat(b, kcl):
            _lab(f"vT_{b}")
            # V natural (k on partitions) via PE transpose of a VT chunk
            g = b * 16 + kcl
            pst = ps_score.tile([P, P], DT_ATTN, tag="sc", name="pst")
            nc.tensor.transpose(pst, VT[:, g * 128:(g + 1) * 128], ident)
            nc.vector.tensor_copy(out=V[:, g, 0:64], in_=pst[:, 0:64])
            nc.vector.tensor_copy(out=V[:, g, 65:129], in_=pst[:, 64:128])

        def scores_exp(b, qt, kcl):
            # scores + exp for one k-chunk; returns the at tile. Emitted
            # ahead of the previous qt's normalize chain, this is PE/ACT
            # work with no dependence on the av accumulators.
            _lab(f"sc_{b}")
            q0 = b * S + qt * 512
            g = b * 16 + kcl
            k0 = b * S + kcl * 128
            sc2 = ps_score.tile([P, 2, 512], dt.float32, tag="sc", name="sc2")
            nc.tensor.matmul(sc2[:, 0, :], lhsT=KT[0:64, k0:k0 + 128],
                             rhs=QT[0:64, q0:q0 + 512])
            nc.tensor.matmul(sc2[:, 1, :], lhsT=KT[64:128, k0:k0 + 128],
                             rhs=QT[64:128, q0:q0 + 512])
            at = apool.tile([P, 2, 512], DT_ATTN, tag="at", name="at")
            bias = mask_sb[:, g:g + 1]
            nc.scalar.activation(at.rearrange("p a n -> p (a n)"),
                                 sc2.rearrange("p a n -> p (a n)"),
                                 mybir.ActivationFunctionType.Exp,
                                 bias=bias, scale=0.125)
            return at

        def av_mms(b, av0, av1, kcl, at):
            _lab(f"av_{b}")
            g = b * 16 + kcl
            sp = (kcl == KC[b] - 1)
            nc.tensor.matmul(av0[0:65], lhsT=V[:, g, 0:65],
                             rhs=at[:, 0, :], start=(kcl == 0), stop=sp)
            nc.tensor.matmul(av1[0:65], lhsT=V[:, g, 65:130],
                             rhs=at[:, 1, :], start=(kcl == 0), stop=sp)

        def attn_chunks(b, qt, av0, av1, kcls, pre=()):
            for i, kcl in enumerate(kcls):
                at = pre[i] if i < len(pre) else scores_exp(b, qt, kcl)
                av_mms(b, av0, av1, kcl, at)

        def avc_copies(av0, av1):
            # drain the av accumulators to SBUF right after the last AV
            # matmul: frees the 2 av PSUM banks for the next q tile and
            # lets the whole normalize chain run deferred (SBUF x PSUM
            # TensorTensor is legal; PSUM x PSUM is not)
            avc0 = rpool.tile([65, 512], DT_ATTN, tag="avc0", name="avc0")
            avc1 = rpool.tile([65, 512], DT_ATTN, tag="avc1", name="avc1")
            nc.scalar.copy(out=avc0, in_=av0[0:65])
            nc.scalar.copy(out=avc1, in_=av1[0:65])
            return avc0, avc1

        def norm_qt(b, qt, avc0, avc1, av0=None, av1=None):
            _lab(f"nrm_{b}")
            q0 = b * S + qt * 512
            # normalize both heads: rowsums at avc0[64] / avc1[64]. 1/r on
            # DVE (fp16, values <= 1), K=1 fp16 matmuls broadcast each 1/r
            # over 64 partitions; DVE multiplies SBUF x PSUM. head1's
            # normalized tile is shifted to partitions 64:128 with a K=64
            # identity matmul (PE is the only partition mover) so Wo
            # contracts both heads in single K=128 matmuls.
            rinv = rpool.tile([P, 2, 512], DT_ATTN, tag="rinv", name="rinv")
            r0src = avc0[64:65, :] if av0 is None else av0[64:65, :]
            r1src = avc1[64:65, :] if av1 is None else av1[64:65, :]
            with nc.allow_low_precision(reason="1/rowsum <= 1 fits fp16"):
                nc.vector.reciprocal(out=rinv[64:65, 0, :], in_=r0src)
                nc.vector.reciprocal(out=rinv[64:65, 1, :], in_=r1src)
            bc0 = ps_pw.tile([P, 512], dt.float32, tag="pw", name="bc0")
            nc.tensor.matmul(bc0[0:64], lhsT=onesk[64:65, :],
                             rhs=rinv[64:65, 0, :])
            nc.vector.tensor_mul(out=outT[0:64, q0:q0 + 512],
                                 in0=avc0[0:64], in1=bc0[0:64])
            bc1 = ps_pw.tile([P, 512], dt.float32, tag="pw", name="bc1")
            nc.tensor.matmul(bc1[0:64], lhsT=onesk[64:65, :],
                             rhs=rinv[64:65, 1, :])
            tmp1 = rpool.tile([DH, 512], DT_ATTN, tag="tmp1", name="tmp1")
            nc.vector.tensor_mul(out=tmp1, in0=avc1[0:64], in1=bc1[0:64])
            mv = ps_pw.tile([P, 512], dt.float32, tag="pw", name="mv")
            nc.tensor.matmul(mv[64:128], lhsT=ident[0:64, 0:64], rhs=tmp1)
            nc.vector.tensor_copy(out=outT[64:128, q0:q0 + 512],
                                  in_=mv[64:128])

        def wo_units(b, qt, last=False):
            # Wo for one q tile as 4 deferred closures (one 128-row s-chunk
            # each: 2 K=128 matmuls + DVE drains + out DMA every 2 chunks).
            # Interleaved into the NEXT q tile's attention so the PSUM
            # drains overlap exp-paced slack instead of stalling PE.
            q0 = b * S + qt * 512
            gs0 = q0 // 128
            ost = opool.tile([P, 4, D], DT_OUT, tag="ost", name="ost")

            def unit(sci):
                def emit():
                    _lab(f"wo_{b}")
                    r0 = (gs0 + sci) * 128
                    for half in range(2):
                        pw = ps_pw.tile([P, 512], dt.float32, tag="pw",
                                        name="pw")
                        w_sl = slice(half * 512, (half + 1) * 512)
                        nc.tensor.matmul(pw, lhsT=outT[:, r0:r0 + 128],
                                         rhs=wall[:, WO_BASE + w_sl.start:
                                                  WO_BASE + w_sl.stop])
                        if last and half == 1:
                            nc.scalar.copy(out=ost[:, sci, w_sl], in_=pw)
                        else:
                            nc.vector.tensor_copy(out=ost[:, sci, w_sl],
                                                  in_=pw)
                    if last:
                        nc.sync.dma_start(
                            out_d.rearrange(
                                "(g p) n -> p g n",
                                p=P)[:, gs0 + sci:gs0 + sci + 1, :],
                            ost[:, sci:sci + 1, :])
                    elif sci == 1 or sci == 3:
                        nc.sync.dma_start(
                            out_d.rearrange(
                                "(g p) n -> p g n",
                                p=P)[:, gs0 + sci - 1:gs0 + sci + 1, :],
                            ost[:, sci - 1:sci + 1, :])
                return emit
            return [unit(s) for s in range(4)]

        # ---- per batch: segmented K/V projection with qt0's attention
        # chunks interleaved, then qt1..3 streaming; Wo work of each qt is
        # deferred into the next qt's attention loop ----
        wo_todo = []

        def attn_seq(b, qt, av0, av1, kcls, i0=0, pre=()):
            for i, kcl in enumerate(kcls):
                at = pre[i] if i < len(pre) else scores_exp(b, qt, kcl)
                av_mms(b, av0, av1, kcl, at)
                if i0 + i >= 2 and wo_todo:
                    wo_todo.pop(0)()

        pending = None  # (b, qt, av0, av1) whose normalize awaits cover

        for b in range(B):
            n_kv_tiles = math.ceil(KC[b] * 128 / 512)
            av0 = ps_av.tile([P, 512], dt.float32, tag="av", name="av0")
            av1 = ps_av.tile([P, 512], dt.float32, tag="av", name="av1")
            for si in range(n_kv_tiles):
                t = b * 4 + si
                g = (b == 0 and si == 0)
                n0 = b * S + si * 512
                wkv = min(512, KC[b] * 128 - si * 512)
                c_lo = si * 4
                c_hi = min(c_lo + 4, KC[b])
                pre0 = ()
                w_sec(WK_BASE)
                if not (b == 1 and si == 0):
                    proj(xtk_d, WK_BASE, [t], KT, granular=g, n0=n0, w=wkv)
                if si == 0:
                    # Q + scores before V: scores need only K and Q, so PE
                    # streams them while V's x tiles are still in flight
                    w_sec(WQ_BASE)
                    proj(xtq_d, WQ_BASE, [b * 4], QT)
                    if b == 0:
                        nc.sync.dma_start(mask_raw, mask_d[:, :])
                        nc.scalar.copy(out=mask_sb, in_=mask_raw)
                    if pending is not None:
                        # b's first segment covers the previous batch's
                        # last normalize/Wo chain
                        pn = pending
                        wo_todo.append(lambda pn=pn: norm_qt(*pn))
                        wo_todo.extend(wo_units(pn[0], pn[1]))
                        pending = None
                    pre0 = [scores_exp(b, 0, kcl)
                            for kcl in range(c_lo, c_hi)]
                w_sec(WV_BASE)
                proj(xtv_d, WV_BASE, [t], VT, n0=n0, w=wkv)
                for kcl in range(c_lo, c_hi):
                    v_nat(b, kcl)
                attn_seq(b, 0, av0, av1, list(range(c_lo, c_hi)), i0=c_lo,
                         pre=pre0)
                if si == 1:
                    w_sec(WO_BASE, D)
                    proj(xtq_d, WQ_BASE, [b * 4 + 1], QT)
            for qt in range(3):
                if qt < 2:
                    # Q projection for qt+2: doubles as cover for this
                    # boundary's normalize chain, and its PSUM drain is
                    # long done before that tile's first scores
                    proj(xtq_d, WQ_BASE, [b * 4 + qt + 2], QT)
                elif b == 0:
                    # next batch's first K tile: covers this boundary and
                    # prefetches its x during b0's last attention tile
                    wkv1 = min(512, KC[1] * 128)
                    proj(xtk_d, WK_BASE, [4], KT, n0=S, w=wkv1)
                c0, c1 = avc_copies(av0, av1)
                wo_todo.append(lambda c0=c0, c1=c1, b=b, qt=qt:
                               norm_qt(b, qt, c0, c1))
                wo_todo.extend(wo_units(b, qt))
                av0 = ps_av.tile([P, 512], dt.float32, tag="av", name="av0")
                av1 = ps_av.tile([P, 512], dt.float32, tag="av", name="av1")
                attn_seq(b, qt + 1, av0, av1, list(range(KC[b])))
            for u in wo_todo:
                u()
            wo_todo = []
            c0, c1 = avc_copies(av0, av1)
            pending = (b, 3, c0, c1)
        if pending is not None:
            norm_qt(*pending)
            for u in wo_units(pending[0], pending[1], last=True):
                u()

    _legalize_waits(nc)
    return nc


def _legalize_waits(nc):
    """This walrus build accepts at most ONE sync-wait command per
    instruction, while Tile emits up to a dozen (e.g. the kernel-tail
    drain). Legalize by splitting: excess waits are hoisted onto
    same-engine Drain instructions inserted immediately before the
    offender — same-engine program order makes this semantically
    identical. Patched module is served via nc.to_json_bytes."""
    import json as _json

    raw = nc.to_json_bytes()
    d = _json.loads(raw)
    template = None
    for fn in d.get("functions", []):
        for blk in fn.get("blocks", []):
            for inst in blk.get("instructions", []):
                if inst.get("opcode") == "Drain":
                    template = inst
                    break
            if template:
                break
        if template:
            break
    assert template is not None, "no Drain template found"

    counter = [0]

    def carrier(engine, wait):
        counter[0] += 1
        c = _json.loads(_json.dumps(template))
        c["name"] = f"I-waitfix-{counter[0]}"
        c["engine"] = engine
        c["sync_info"] = {"on_update": [], "on_wait": [wait]}
        c["ins"] = []
        c["outs"] = []
        return c

    nfix = 0
    for fn in d.get("functions", []):
        for blk in fn.get("blocks", []):
            out = []
            for inst in blk.get("instructions", []):
                si = inst.get("sync_info")
                waits = (si or {}).get("on_wait") or []
                if len(waits) > 1:
                    for w in waits[:-1]:
                        out.append(carrier(inst["engine"], w))
                    si["on_wait"] = [waits[-1]]
                    nfix += 1
                out.append(inst)
            blk["instructions"] = out

    patched = _json.dumps(d).encode()
    nc.to_json_bytes = lambda: patched


def _prep_host(queries, keys, values, Wq, Wk, Wv, Wo, valid_lens, cfg):
    np_in = {"bfloat16": ml_dtypes.bfloat16, "float32": np.float32,
             "float32r": np.float32, "float16": np.float16}[cfg["dt_in"]]
    L = [int(valid_lens[0]), int(valid_lens[1])]
    KC = tuple(min(16, (l + 127) // 128) for l in L)

    def t2(x):  # (B,S,D) -> (D, B*S)
        return np.ascontiguousarray(
            np.asarray(x, np.float32).reshape(NT, D).T).astype(np_in)

    xtq, xtk, xtv = t2(queries), t2(keys), t2(values)
    maskt = np.full((P, B * 16), NEG, np.float32)
    for b in range(B):
        for c in range(16):
            ks = c * 128 + np.arange(P)
            maskt[:, b * 16 + c] = np.where(ks < L[b], 0.0, NEG)

    Wq = np.asarray(Wq, np.float32)
    Wk = np.asarray(Wk, np.float32)
    Wv = np.asarray(Wv, np.float32)
    Wo = np.asarray(Wo, np.float32)

    def packw(Wx, cs):  # [D, F] slice -> [P, DK*F], p-major rows (2KB elems)
        return Wx[:, cs].reshape(DK, P, F).transpose(1, 0, 2).reshape(P, DK * F)

    in_maps = []
    for c in range(N_CORES):
        cs = slice(c * F, (c + 1) * F)
        wall = np.concatenate(
            [packw(Wq, cs), packw(Wk, cs), packw(Wv, cs), Wo[cs, :]],
            axis=1)
        in_maps.append({
            "xtq": xtq, "xtk": xtk, "xtv": xtv,
            "wall": np.ascontiguousarray(wall).astype(np_in),
            "maskt": maskt,
        })
    return KC, in_maps


DEFAULT_CFG = {"dt_in": "float16", "dt_attn": "float16", "dt_out": "float16"}

LAST_RESULTS = None


def kernel(queries, keys, values, Wq, Wk, Wv, Wo, valid_lens):
    global LAST_RESULTS
    from concourse.bass_utils import run_bass_kernel_spmd

    cfg = dict(DEFAULT_CFG)
    if os.environ.get("MHA_CFG"):
        for kv in os.environ["MHA_CFG"].split(","):
            k, v = kv.split("=")
            cfg[k] = v

    KC, in_maps = _prep_host(queries, keys, values, Wq, Wk, Wv, Wo,
                             valid_lens, cfg)
    key = (KC, tuple(sorted(cfg.items())))
    if key not in _CACHE:
        _CACHE[key] = _build_program(KC, cfg)
    nc = _CACHE[key]

    trace = bool(os.environ.get("MHA_TRACE"))
    res = run_bass_kernel_spmd(nc, in_maps, core_ids=list(range(N_CORES)),
                               trace=trace)
    LAST_RESULTS = res
    acc = np.zeros((NT, D), np.float32)
    for r in res.results:
        acc += np.asarray(r["out_part"], np.float32)
    return acc.reshape(B, S, D)
